# revision 23
# baseline (speedup 1.0000x reference)
"""Trainium2 Bass kernel for a 2-stage 13-organ Dice loss.

Math (all organ weights are 1.0, so the per-organ fold collapses to sums):
  for stage s, batch b:
    num[s,b] = 2 * sum_{c in 1..13} sum_v pred_s[b,c,v] * [target[b,v]==c]
    den[s,b] = sum_{c in 1..13} sum_v pred_s[b,c,v]^2 + count(target[b]!=0) + 13*EPS
  dice[b] = num[1,b]/den[1,b] + num[2,b]/den[2,b]
  loss    = mean_b(2 - dice[b])

Layout strategy (chosen for the memory-bound regime):
  * pred is cast to fp8-e4m3 on the host (device sees float8e4). The loss is
    a ratio of sums over ~40M elements, so the zero-mean fp8 rounding noise
    averages down to ~2e-4 relative on the final scalar (tolerance 2e-2).
  * Voxels are SORTED BY TARGET CLASS on the host (per batch), each class run
    padded with zero voxels to a multiple of 8*1024, and round-robined across
    the 8 cores so every core gets an identical per-class unit structure
    (same SPMD program).  A "unit" is 1024 voxels laid out as [128 part x 8].
  * Because each 1024-voxel unit is single-class, the one-hot mask over a
    unit is all-ones, so the numerator needs NO mask tensors and NO per-chunk
    stationary loads: it is a plain column-sum matmul against a constant
    ones stationary, accumulated into PSUM per (stage, batch).
  * The denominator sum-of-squares is split across three engines by channel
    slot so every engine finishes in ~the DMA time:
      slots 0..5  -> PE   (diagonal trick: matmul(chunk, chunk) accumulated
                           into PSUM; host extracts the trace)
      slots 6..9  -> ACT  (activation Square with fused accumulator)
      slots 10..12-> DVE  (scalar_tensor_tensor mult with fused accumulator)
  * count(target != 0) comes from an ACT Sign pass (sign(t) sums to the
    nonzero count) over the sorted fp8 target slab.
  * Host does the final tiny reduction across cores and the dice division.
"""

import numpy as np
import ml_dtypes

import concourse.bacc as bacc
import concourse.mybir as mybir
import concourse.tile as tile
from concourse.bass_utils import run_bass_kernel_spmd

N_CORES = 8
S = 2            # stages
B = 2            # batch
C = 13           # organ channels (pred channels 1..13; channel 0 dropped)
NCLS = 14        # target classes 0..13 (0 = background)
D, H, W = 48, 256, 256
NV = D * H * W   # voxels per batch element
UNIT = 1024      # voxels per unit = [128 partitions x 8 cols]
UJ = UNIT // 128  # 8 cols per unit
TILE_G = 112     # units per DMA tile
EPS = 1e-5

# den channel-slot split (slots are pred channels 1..13 minus 1)
PE_SLOTS = range(0, 7)
ACT_SLOTS = (7, 10)    # slice [7,10)
DVE_SLOTS = (10, 13)   # slice [10,13)

F32 = mybir.dt.float32
FP8 = mybir.dt.float8e4
NP_FP8 = ml_dtypes.float8_e4m3


def _plan(counts_b):
    """Static per-core plan from per-(b,class) voxel counts.

    Returns dict with per-b: units-per-class, tile sizes, per-tile num
    segments (slot, tile-local col0, ncols), and offsets."""
    plan = {"b": []}
    for b in range(B):
        counts = counts_b[b]
        k = [int(-(-int(counts[c]) // (N_CORES * UNIT))) for c in range(NCLS)]
        U = sum(k)
        # graduated tile sizes: small tiles at both ends so the DMA->compute
        # pipeline fills fast and the tail compute quantum is short
        tgs = []
        rem = U
        for want in (28, 56):
            if rem > want:
                tgs.append(want)
                rem -= want
        while rem > TILE_G + 84:
            tgs.append(TILE_G)
            rem -= TILE_G
        for want in (84, 56, 28):
            if rem > want:
                tgs.append(want)
                rem -= want
        if rem:
            tgs.append(rem)
        # class run of unit u (global per-core unit index)
        cls_of_unit = np.repeat(np.arange(NCLS), k)
        # tile start units
        tstart = np.concatenate([[0], np.cumsum(tgs)])
        # num segments per tile: maximal same-class runs, classes >= 1
        segs = [[] for _ in tgs]
        maxn = 0
        u0 = 0
        for cls in range(NCLS):
            if k[cls] == 0:
                continue
            u1 = u0 + k[cls]
            if cls >= 1:
                # split [u0, u1) on tile boundaries, and cap each num matmul
                # at 384 columns so it never reaches the den-diag region
                # (cols 384:512) of the shared per-(s,b) PSUM bank
                a = u0
                while a < u1:
                    t = int(np.searchsorted(tstart, a, side="right")) - 1
                    t_end = min(u1, int(tstart[t + 1]), a + 384 // UJ)
                    ncols = (t_end - a) * UJ
                    segs[t].append((cls - 1, (a - int(tstart[t])) * UJ, ncols))
                    maxn = max(maxn, ncols)
                    a = t_end
            u0 = u1
        plan["b"].append(
            dict(
                k=k,
                U=U,
                tgs=tgs,
                segs=segs,
                maxn=maxn,
                cls_of_unit=cls_of_unit,
            )
        )
    return plan


def build_program(plan):
    nc = bacc.Bacc(target_bir_lowering=False)
    # one contiguous dram tensor per (b, tile) so every big DMA reads one
    # dense HBM block
    pred = {
        (b, t): nc.dram_tensor(
            f"pred_{b}_{t}", [128, S * C * tg_u * UJ], FP8, kind="ExternalInput"
        )
        for b in range(B)
        for t, tg_u in enumerate(plan["b"][b]["tgs"])
    }
    tgt = {
        b: nc.dram_tensor(
            f"tgt_{b}", [128, plan["b"][b]["U"] * UJ], FP8, kind="ExternalInput"
        )
        for b in range(B)
    }
    # single packed output: [psum dumps (4x512) | act slots 32 | dve slots 32
    # | cnt slots 4]
    out = nc.dram_tensor("out", [128, S * B * 512 + 68], F32, kind="ExternalOutput")

    n_act = ACT_SLOTS[1] - ACT_SLOTS[0]
    n_dve = DVE_SLOTS[1] - DVE_SLOTS[0]
    n_pe = len(PE_SLOTS)

    # All matmuls of one (s, b) — den-diag chunks AND num column sums — form
    # a single PSUM accumulation group in one exclusive bank: a start=True
    # clears has_written for the WHOLE bank, so each bank must see exactly
    # one start.  num lives in cols [0:384), den-diag in cols [384:512).
    mm_total = {}
    for b in range(B):
        pb = plan["b"][b]
        nchunk = sum(-(-tg * UJ // 128) for tg in pb["tgs"]) * n_pe
        nseg = sum(len(s) for s in pb["segs"])
        for s in range(S):
            mm_total[(s, b)] = nchunk + nseg

    with tile.TileContext(nc) as tc:
        with (
            tc.tile_pool(name="pt", bufs=4) as ppool,
            tc.tile_pool(name="tg", bufs=1) as tpool,
            tc.tile_pool(name="scr", bufs=1) as spool,
            tc.tile_pool(name="ps", bufs=1, space="PSUM") as qpool,
        ):
            ones = spool.tile([128, 128], FP8, tag="ones")
            nc.vector.memset(ones[:, :], 1.0)
            outb = spool.tile([128, S * B * 512 + 68], F32, tag="outb")
            act_slots = outb[:, 2048:2080]
            dve_slots = outb[:, 2080:2112]
            cnt_slots = outb[:, 2112:2116]
            nc.vector.memset(outb[:, 2048:], 0.0)
            adummy = spool.tile([128, n_act * TILE_G * UJ], FP8, tag="ad")
            vdummy = spool.tile([128, n_dve * TILE_G * UJ], FP8, tag="vd")
            cdummy = spool.tile([128, max(p["U"] for p in plan["b"]) * UJ], FP8, tag="cd")

            ps = {
                (s, b): qpool.tile([128, 512], F32, tag=f"pn{s}{b}", name=f"pn{s}{b}")
                for s in range(S)
                for b in range(B)
            }
            mm_ct = {k: 0 for k in mm_total}

            slot_i = 0
            for b in range(B):
                pb = plan["b"][b]
                U = pb["U"]
                # whole-b sorted target slab; counts nonzeros via ACT Sign
                tb = tpool.tile([128, U * UJ], FP8, tag=f"tb{b}")
                nc.sync.dma_start(out=tb[:, :], in_=tgt[b][:, :])
                nc.scalar.activation(
                    cdummy[:, : U * UJ],
                    tb[:, :],
                    mybir.ActivationFunctionType.Sign,
                    accum_out=cnt_slots[:, b : b + 1],
                )
                for t, tg_u in enumerate(pb["tgs"]):
                    L = tg_u * UJ  # cols per (s, c) in this tile
                    pt = ppool.tile([128, S, C, L], FP8, tag="pt")
                    nc.sync.dma_start(
                        out=pt[:, :, :, :],
                        in_=pred[(b, t)][:, :],
                    )
                    for s in range(S):
                        # ACT den slots
                        nc.scalar.activation(
                            adummy[:, : n_act * L],
                            pt[:, s, ACT_SLOTS[0] : ACT_SLOTS[1], :],
                            mybir.ActivationFunctionType.Square,
                            accum_out=act_slots[:, slot_i : slot_i + 1],
                        )
                        # DVE den slots
                        nc.vector.scalar_tensor_tensor(
                            out=vdummy[:, : n_dve * L],
                            in0=pt[:, s, DVE_SLOTS[0] : DVE_SLOTS[1], :],
                            scalar=1.0,
                            in1=pt[:, s, DVE_SLOTS[0] : DVE_SLOTS[1], :],
                            op0=mybir.AluOpType.mult,
                            op1=mybir.AluOpType.mult,
                            accum_out=dve_slots[:, slot_i : slot_i + 1],
                        )
                        # PE den slots: diagonal-trick chunks -> cols 384:512
                        pn = ps[(s, b)]
                        for c in PE_SLOTS:
                            for k0 in range(0, L, 128):
                                w = min(128, L - k0)
                                ch = pt[:, s, c, k0 : k0 + w]
                                mm_ct[(s, b)] += 1
                                nc.tensor.matmul(
                                    pn[:w, 384 : 384 + w],
                                    ch,
                                    ch,
                                    start=(mm_ct[(s, b)] == 1),
                                    stop=(mm_ct[(s, b)] == mm_total[(s, b)]),
                                )
                        # numerator: ones-stationary column sums per segment
                        for slot, col0, ncols in pb["segs"][t]:
                            mm_ct[(s, b)] += 1
                            nc.tensor.matmul(
                                pn[:, :ncols],
                                ones[:, :],
                                pt[:, s, slot, col0 : col0 + ncols],
                                start=(mm_ct[(s, b)] == 1),
                                stop=(mm_ct[(s, b)] == mm_total[(s, b)]),
                            )
                        slot_i += 1

            # extract psums -> sbuf (split across DVE and ACT) -> one dram DMA
            for s in range(S):
                for b in range(B):
                    q = b * S + s
                    dst = outb[:, q * 512 : (q + 1) * 512]
                    if q % 2 == 0:
                        nc.vector.tensor_copy(dst, ps[(s, b)][:, :])
                    else:
                        nc.scalar.copy(dst, ps[(s, b)][:, :])
            nc.sync.dma_start(out=out[:, :], in_=outb[:, :])
    nc.finalize()
    return nc


def shard_inputs(pred_stage1, pred_stage2, target):
    """Sort voxels by class, pad class runs, split across cores, pack fp8."""
    p1 = np.asarray(pred_stage1)
    p2 = np.asarray(pred_stage2)
    tg = np.asarray(target)
    counts_b = []
    orders = []
    for b in range(B):
        t = tg[b].reshape(-1)
        orders.append(np.argsort(t, kind="stable"))
        counts_b.append(np.bincount(t.astype(np.int64), minlength=NCLS))
    plan = _plan(counts_b)

    # fp8 quantized pred, channels 1..13 only: [S, C, NV] per b
    pq = [
        np.stack(
            [
                np.asarray(p1[b, 1:]).reshape(C, NV).astype(NP_FP8),
                np.asarray(p2[b, 1:]).reshape(C, NV).astype(NP_FP8),
            ]
        )
        for b in range(B)
    ]

    in_maps = [{} for _ in range(N_CORES)]
    for b in range(B):
        pb = plan["b"][b]
        counts = counts_b[b]
        U = pb["U"]
        k = pb["k"]
        order = orders[b]
        # global per-class padded index arrays -> per-core [U, 128, UJ]
        vidx_cores = np.full((N_CORES, U, 128, UJ), -1, np.int64)
        pos = 0
        u0 = 0
        for cls in range(NCLS):
            n = int(counts[cls])
            if k[cls] == 0:
                continue
            P = k[cls] * N_CORES * UNIT
            idx = np.full(P, -1, np.int64)
            idx[:n] = order[pos : pos + n]
            pos += n
            vidx_cores[:, u0 : u0 + k[cls]] = idx.reshape(
                N_CORES, k[cls], 128, UJ
            )
            u0 += k[cls]
        cls_units = pb["cls_of_unit"]  # [U]
        for core in range(N_CORES):
            vidx = vidx_cores[core]  # [U, 128, UJ]
            valid = vidx >= 0
            vclip = np.where(valid, vidx, 0)
            # target slab [128, U*UJ]
            tval = np.where(valid, cls_units[:, None, None], 0).astype(NP_FP8)
            in_maps[core][f"tgt_{b}"] = np.ascontiguousarray(
                tval.transpose(1, 0, 2).reshape(128, U * UJ)
            )
            # pred gather: [S, C, U, 128, UJ]
            g = pq[b][:, :, vclip]
            g = np.where(valid[None, None], g, NP_FP8(0))
            t0 = 0
            for t, tg_u in enumerate(pb["tgs"]):
                blk = g[:, :, t0 : t0 + tg_u]  # [S, C, tg_u, 128, UJ]
                blk = np.ascontiguousarray(
                    blk.transpose(3, 0, 1, 2, 4).reshape(128, -1)
                )
                in_maps[core][f"pred_{b}_{t}"] = blk
                t0 += tg_u
    return in_maps, plan


def combine_results(results, plan):
    num = np.zeros((S, B), np.float64)
    den = np.zeros((S, B), np.float64)
    cnt = np.zeros(B, np.float64)
    for r in results:
        o = r["out"].astype(np.float64)
        oden = o[:, :2048]
        oact = o[:, 2048:2080]
        odve = o[:, 2080:2112]
        ocnt = o[:, 2112:2116]
        slot_i = 0
        for b in range(B):
            pb = plan["b"][b]
            cnt[b] += ocnt[:, b].sum()
            for s in range(S):
                q = b * S + s
                blk = oden[:, q * 512 : (q + 1) * 512]
                num[s, b] += blk[0, : pb["maxn"]].sum()
                den[s, b] += np.trace(blk[:, 384:512])
        for b in range(B):
            pb = plan["b"][b]
            for t in range(len(pb["tgs"])):
                for s in range(S):
                    den[s, b] += oact[:, slot_i].sum() + odve[:, slot_i].sum()
                    slot_i += 1
    dice = np.zeros(B, np.float64)
    for b in range(B):
        for s in range(S):
            dice[b] += 2.0 * num[s, b] / (den[s, b] + cnt[b] + C * EPS)
    loss = np.mean(2.0 - dice)
    return np.array(loss, dtype=np.float32)


def kernel(pred_stage1, pred_stage2, target):
    in_maps, plan = shard_inputs(pred_stage1, pred_stage2, target)
    nc = build_program(plan)
    # The first multi-core execution of a freshly loaded NEFF occasionally
    # hits a transient NRT_EXEC_UNIT_UNRECOVERABLE; a retry succeeds.
    last_err = None
    for _ in range(3):
        try:
            res = run_bass_kernel_spmd(nc, in_maps, list(range(N_CORES)))
            return combine_results(res.results, plan)
        except Exception as e:  # noqa: BLE001
            last_err = e
    raise last_err


# revision 26
# speedup vs baseline: 1.1037x; 1.1037x over previous
"""Trainium2 Bass kernel for a 2-stage 13-organ Dice loss.

Math (all organ weights are 1.0, so the per-organ fold collapses to sums):
  for stage s, batch b:
    num[s,b] = 2 * sum_{c in 1..13} sum_v pred_s[b,c,v] * [target[b,v]==c]
    den[s,b] = sum_{c in 1..13} sum_v pred_s[b,c,v]^2 + count(target[b]!=0) + 13*EPS
  dice[b] = num[1,b]/den[1,b] + num[2,b]/den[2,b]
  loss    = mean_b(2 - dice[b])

Layout strategy (chosen for the memory-bound regime):
  * pred is cast to fp8-e4m3 on the host (device sees float8e4). The loss is
    a ratio of sums over ~40M elements, so the zero-mean fp8 rounding noise
    averages down to ~2e-4 relative on the final scalar (tolerance 2e-2).
  * Voxels are SORTED BY TARGET CLASS on the host (per batch), each class run
    padded with zero voxels to a multiple of 8*1024, and round-robined across
    the 8 cores so every core gets an identical per-class unit structure
    (same SPMD program).  A "unit" is 1024 voxels laid out as [128 part x 8].
  * Because each 1024-voxel unit is single-class, the one-hot mask over a
    unit is all-ones, so the numerator needs NO mask tensors and NO per-chunk
    stationary loads: it is a plain column-sum matmul against a constant
    ones stationary, accumulated into PSUM per (stage, batch).
  * The denominator sum-of-squares is split across three engines by channel
    slot so every engine finishes in ~the DMA time:
      slots 0..5  -> PE   (diagonal trick: matmul(chunk, chunk) accumulated
                           into PSUM; host extracts the trace)
      slots 6..9  -> ACT  (activation Square with fused accumulator)
      slots 10..12-> DVE  (scalar_tensor_tensor mult with fused accumulator)
  * count(target != 0) comes from an ACT Sign pass (sign(t) sums to the
    nonzero count) over the sorted fp8 target slab.
  * Host does the final tiny reduction across cores and the dice division.
"""

import numpy as np
import ml_dtypes

import concourse.bacc as bacc
import concourse.mybir as mybir
import concourse.tile as tile
from concourse.bass_utils import run_bass_kernel_spmd

N_CORES = 8
S = 2            # stages
B = 2            # batch
C = 13           # organ channels (pred channels 1..13; channel 0 dropped)
NCLS = 14        # target classes 0..13 (0 = background)
D, H, W = 48, 256, 256
NV = D * H * W   # voxels per batch element
UNIT = 1024      # voxels per unit = [128 partitions x 8 cols]
UJ = UNIT // 128  # 8 cols per unit
TILE_G = 112     # units per DMA tile
EPS = 1e-5

# den channel-slot split (slots are pred channels 1..13 minus 1)
PE_SLOTS = range(0, 6)
ACT_SLOTS = (6, 10)    # slice [6,10)
DVE_SLOTS = (10, 13)   # slice [10,13)

F32 = mybir.dt.float32
FP8 = mybir.dt.float8e4
NP_FP8 = ml_dtypes.float8_e4m3


def _plan(counts_b):
    """Static per-core plan from per-(b,class) voxel counts.

    Returns dict with per-b: units-per-class, tile sizes, per-tile num
    segments (slot, tile-local col0, ncols), and offsets."""
    plan = {"b": []}
    for b in range(B):
        counts = counts_b[b]
        k = [int(-(-int(counts[c]) // (N_CORES * UNIT))) for c in range(NCLS)]
        U = sum(k)
        # graduated tile sizes: a smaller first tile fills the DMA->compute
        # pipeline fast, small last tiles keep the drain tail short
        tgs = []
        rem = U
        if rem > 56:
            tgs.append(56)
            rem -= 56
        while rem > TILE_G + 56:
            tgs.append(TILE_G)
            rem -= TILE_G
        for want in (56, 28):
            if rem > want:
                tgs.append(want)
                rem -= want
        if rem:
            tgs.append(rem)
        # class run of unit u (global per-core unit index)
        cls_of_unit = np.repeat(np.arange(NCLS), k)
        # tile start units
        tstart = np.concatenate([[0], np.cumsum(tgs)])
        # num segments per tile: maximal same-class runs, classes >= 1
        segs = [[] for _ in tgs]
        maxn = 0
        u0 = 0
        for cls in range(NCLS):
            if k[cls] == 0:
                continue
            u1 = u0 + k[cls]
            if cls >= 1:
                # split [u0, u1) on tile boundaries, and cap each num matmul
                # at 384 columns so it never reaches the den-diag region
                # (cols 384:512) of the shared per-(s,b) PSUM bank
                a = u0
                while a < u1:
                    t = int(np.searchsorted(tstart, a, side="right")) - 1
                    t_end = min(u1, int(tstart[t + 1]), a + 384 // UJ)
                    ncols = (t_end - a) * UJ
                    segs[t].append((cls - 1, (a - int(tstart[t])) * UJ, ncols))
                    maxn = max(maxn, ncols)
                    a = t_end
            u0 = u1
        plan["b"].append(
            dict(
                k=k,
                U=U,
                tgs=tgs,
                segs=segs,
                maxn=maxn,
                cls_of_unit=cls_of_unit,
            )
        )
    return plan


def build_program(plan):
    nc = bacc.Bacc(target_bir_lowering=False)
    # one contiguous dram tensor per (b, tile) so every big DMA reads one
    # dense HBM block
    pred = {
        (b, t): nc.dram_tensor(
            f"pred_{b}_{t}", [128, S * C * tg_u * UJ], FP8, kind="ExternalInput"
        )
        for b in range(B)
        for t, tg_u in enumerate(plan["b"][b]["tgs"])
    }
    tgt = {
        b: nc.dram_tensor(
            f"tgt_{b}", [128, plan["b"][b]["U"] * UJ], FP8, kind="ExternalInput"
        )
        for b in range(B)
    }
    # single packed output: [psum dumps (4x512) | act slots 32 | dve slots 32
    # | cnt slots 4]
    out = nc.dram_tensor("out", [128, S * B * 512 + 68], F32, kind="ExternalOutput")

    n_act = ACT_SLOTS[1] - ACT_SLOTS[0]
    n_dve = DVE_SLOTS[1] - DVE_SLOTS[0]
    n_pe = len(PE_SLOTS)

    # All matmuls of one (s, b) — den-diag chunks AND num column sums — form
    # a single PSUM accumulation group in one exclusive bank: a start=True
    # clears has_written for the WHOLE bank, so each bank must see exactly
    # one start.  num lives in cols [0:384), den-diag in cols [384:512).
    mm_total = {}
    for b in range(B):
        pb = plan["b"][b]
        nchunk = sum(-(-tg * UJ // 128) for tg in pb["tgs"]) * n_pe
        nseg = sum(len(s) for s in pb["segs"])
        for s in range(S):
            mm_total[(s, b)] = nchunk + nseg

    with tile.TileContext(nc) as tc:
        with (
            tc.tile_pool(name="pt", bufs=4) as ppool,
            tc.tile_pool(name="tg", bufs=1) as tpool,
            tc.tile_pool(name="scr", bufs=1) as spool,
            tc.tile_pool(name="ps", bufs=1, space="PSUM") as qpool,
        ):
            ones = spool.tile([128, 128], FP8, tag="ones")
            nc.vector.memset(ones[:, :], 1.0)
            outb = spool.tile([128, S * B * 512 + 68], F32, tag="outb")
            act_slots = outb[:, 2048:2080]
            dve_slots = outb[:, 2080:2112]
            cnt_slots = outb[:, 2112:2116]
            nc.vector.memset(outb[:, 2048:], 0.0)
            adummy = spool.tile([128, n_act * TILE_G * UJ], FP8, tag="ad")
            vdummy = spool.tile([128, n_dve * TILE_G * UJ], FP8, tag="vd")
            cdummy = spool.tile([128, max(p["U"] for p in plan["b"]) * UJ], FP8, tag="cd")

            ps = {
                (s, b): qpool.tile([128, 512], F32, tag=f"pn{s}{b}", name=f"pn{s}{b}")
                for s in range(S)
                for b in range(B)
            }
            mm_ct = {k: 0 for k in mm_total}

            slot_i = 0
            for b in range(B):
                pb = plan["b"][b]
                U = pb["U"]
                # whole-b sorted target slab; counts nonzeros via ACT Sign
                tb = tpool.tile([128, U * UJ], FP8, tag=f"tb{b}")
                nc.sync.dma_start(out=tb[:, :], in_=tgt[b][:, :])
                nc.scalar.activation(
                    cdummy[:, : U * UJ],
                    tb[:, :],
                    mybir.ActivationFunctionType.Sign,
                    accum_out=cnt_slots[:, b : b + 1],
                )
                for t, tg_u in enumerate(pb["tgs"]):
                    L = tg_u * UJ  # cols per (s, c) in this tile
                    pt = ppool.tile([128, S, C, L], FP8, tag="pt")
                    nc.sync.dma_start(
                        out=pt[:, :, :, :],
                        in_=pred[(b, t)][:, :],
                    )
                    for s in range(S):
                        # ACT den slots
                        nc.scalar.activation(
                            adummy[:, : n_act * L],
                            pt[:, s, ACT_SLOTS[0] : ACT_SLOTS[1], :],
                            mybir.ActivationFunctionType.Square,
                            accum_out=act_slots[:, slot_i : slot_i + 1],
                        )
                        # DVE den slots
                        nc.vector.scalar_tensor_tensor(
                            out=vdummy[:, : n_dve * L],
                            in0=pt[:, s, DVE_SLOTS[0] : DVE_SLOTS[1], :],
                            scalar=1.0,
                            in1=pt[:, s, DVE_SLOTS[0] : DVE_SLOTS[1], :],
                            op0=mybir.AluOpType.mult,
                            op1=mybir.AluOpType.mult,
                            accum_out=dve_slots[:, slot_i : slot_i + 1],
                        )
                        # PE den slots: diagonal-trick chunks -> cols 384:512
                        pn = ps[(s, b)]
                        for c in PE_SLOTS:
                            for k0 in range(0, L, 128):
                                w = min(128, L - k0)
                                ch = pt[:, s, c, k0 : k0 + w]
                                mm_ct[(s, b)] += 1
                                nc.tensor.matmul(
                                    pn[:w, 384 : 384 + w],
                                    ch,
                                    ch,
                                    start=(mm_ct[(s, b)] == 1),
                                    stop=(mm_ct[(s, b)] == mm_total[(s, b)]),
                                )
                        # numerator: ones-stationary column sums per segment
                        for slot, col0, ncols in pb["segs"][t]:
                            mm_ct[(s, b)] += 1
                            nc.tensor.matmul(
                                pn[:, :ncols],
                                ones[:, :],
                                pt[:, s, slot, col0 : col0 + ncols],
                                start=(mm_ct[(s, b)] == 1),
                                stop=(mm_ct[(s, b)] == mm_total[(s, b)]),
                            )
                        slot_i += 1

                # this b's psum groups are complete: extract now (overlaps
                # the next b's stream), split across DVE and ACT
                for s in range(S):
                    q = b * S + s
                    dst = outb[:, q * 512 : (q + 1) * 512]
                    if s == 0:
                        nc.vector.tensor_copy(dst, ps[(s, b)][:, :])
                    else:
                        nc.scalar.copy(dst, ps[(s, b)][:, :])
                if b == 0:
                    nc.sync.dma_start(
                        out=out[:, : S * 512], in_=outb[:, : S * 512]
                    )
            nc.sync.dma_start(out=out[:, S * 512 :], in_=outb[:, S * 512 :])
    nc.finalize()
    return nc


def shard_inputs(pred_stage1, pred_stage2, target):
    """Sort voxels by class, pad class runs, split across cores, pack fp8."""
    p1 = np.asarray(pred_stage1)
    p2 = np.asarray(pred_stage2)
    tg = np.asarray(target)
    counts_b = []
    orders = []
    for b in range(B):
        t = tg[b].reshape(-1)
        orders.append(np.argsort(t, kind="stable"))
        counts_b.append(np.bincount(t.astype(np.int64), minlength=NCLS))
    plan = _plan(counts_b)

    # fp8 quantized pred, channels 1..13 only: [S, C, NV] per b
    pq = [
        np.stack(
            [
                np.asarray(p1[b, 1:]).reshape(C, NV).astype(NP_FP8),
                np.asarray(p2[b, 1:]).reshape(C, NV).astype(NP_FP8),
            ]
        )
        for b in range(B)
    ]

    in_maps = [{} for _ in range(N_CORES)]
    for b in range(B):
        pb = plan["b"][b]
        counts = counts_b[b]
        U = pb["U"]
        k = pb["k"]
        order = orders[b]
        # global per-class padded index arrays -> per-core [U, 128, UJ]
        vidx_cores = np.full((N_CORES, U, 128, UJ), -1, np.int64)
        pos = 0
        u0 = 0
        for cls in range(NCLS):
            n = int(counts[cls])
            if k[cls] == 0:
                continue
            P = k[cls] * N_CORES * UNIT
            idx = np.full(P, -1, np.int64)
            idx[:n] = order[pos : pos + n]
            pos += n
            vidx_cores[:, u0 : u0 + k[cls]] = idx.reshape(
                N_CORES, k[cls], 128, UJ
            )
            u0 += k[cls]
        cls_units = pb["cls_of_unit"]  # [U]
        for core in range(N_CORES):
            vidx = vidx_cores[core]  # [U, 128, UJ]
            valid = vidx >= 0
            vclip = np.where(valid, vidx, 0)
            # target slab [128, U*UJ]
            tval = np.where(valid, cls_units[:, None, None], 0).astype(NP_FP8)
            in_maps[core][f"tgt_{b}"] = np.ascontiguousarray(
                tval.transpose(1, 0, 2).reshape(128, U * UJ)
            )
            # pred gather: [S, C, U, 128, UJ]
            g = pq[b][:, :, vclip]
            g = np.where(valid[None, None], g, NP_FP8(0))
            t0 = 0
            for t, tg_u in enumerate(pb["tgs"]):
                blk = g[:, :, t0 : t0 + tg_u]  # [S, C, tg_u, 128, UJ]
                blk = np.ascontiguousarray(
                    blk.transpose(3, 0, 1, 2, 4).reshape(128, -1)
                )
                in_maps[core][f"pred_{b}_{t}"] = blk
                t0 += tg_u
    return in_maps, plan


def combine_results(results, plan):
    num = np.zeros((S, B), np.float64)
    den = np.zeros((S, B), np.float64)
    cnt = np.zeros(B, np.float64)
    for r in results:
        o = r["out"].astype(np.float64)
        oden = o[:, :2048]
        oact = o[:, 2048:2080]
        odve = o[:, 2080:2112]
        ocnt = o[:, 2112:2116]
        slot_i = 0
        for b in range(B):
            pb = plan["b"][b]
            cnt[b] += ocnt[:, b].sum()
            for s in range(S):
                q = b * S + s
                blk = oden[:, q * 512 : (q + 1) * 512]
                num[s, b] += blk[0, : pb["maxn"]].sum()
                den[s, b] += np.trace(blk[:, 384:512])
        for b in range(B):
            pb = plan["b"][b]
            for t in range(len(pb["tgs"])):
                for s in range(S):
                    den[s, b] += oact[:, slot_i].sum() + odve[:, slot_i].sum()
                    slot_i += 1
    dice = np.zeros(B, np.float64)
    for b in range(B):
        for s in range(S):
            dice[b] += 2.0 * num[s, b] / (den[s, b] + cnt[b] + C * EPS)
    loss = np.mean(2.0 - dice)
    return np.array(loss, dtype=np.float32)


def kernel(pred_stage1, pred_stage2, target):
    in_maps, plan = shard_inputs(pred_stage1, pred_stage2, target)
    nc = build_program(plan)
    # The first multi-core execution of a freshly loaded NEFF occasionally
    # hits a transient NRT_EXEC_UNIT_UNRECOVERABLE; a retry succeeds.
    last_err = None
    for _ in range(3):
        try:
            res = run_bass_kernel_spmd(nc, in_maps, list(range(N_CORES)))
            return combine_results(res.results, plan)
        except Exception as e:  # noqa: BLE001
            last_err = e
    raise last_err
